# revision 47
# baseline (speedup 1.0000x reference)
"""Trainium2 Bass kernel for nn_EquivariantBackbone (e3nn-style equivariant GNN).

Strategy (8 NeuronCores, SPMD):
  - Edges sharded across cores (256 edges/core); node features replicated.
  - Per-edge radial weights are never materialized: per conv and l1-block the
    contraction  z[e,:] = sum_{t,u} h[e,t] * x1[e,u,i] * w2[t,u,:]  runs as
    nt PSUM-accumulated matmuls with lhsT = G_t = x1T * broadcast(h[:,t]) and
    rhs = the (t,u)-major w2 slab -- full-K PE matmuls, no K=12 waste.
  - Wigner/spherical coefficient contraction (i->k) folded into per-edge
    scalar columns s = sh @ Cmat (Cmat is a host constant), applied with
    fused scalar_tensor_tensor ops (e on partitions).
  - Scatter-add onto source nodes via an on-chip one-hot incidence matmul
    (S built from iota + is_equal against src indices, contraction over e);
    multiple message blocks are stacked along output partitions per matmul.
  - Partial node aggregates AllReduced (fp16) across the 8 cores; node phase
    (norm / self-interaction / gated nonlinearity) replicated on all cores.
  - x[dst] gathers for conv2/3 via one-hot incidence matmuls.
  - softplus built as -ln(sigmoid(-y)) (no softplus in this act table); the
    sign is folded into consumers (negated transpose identity / final w).
  - ALL inputs are packed into two per-core blob tensors (blob16/blob32):
    per-dispatch buffer-binding overhead on the axon PJRT path is ~0.3 ms
    per input tensor, so 35 separate inputs cost ~11 ms of pure dispatch.
    Constants are loaded with 8 grouped DMAs; per-run instruction count is
    the dominant device-side cost on this runtime.

kernel(**inputs) accepts the full unsharded inputs, returns (512, 32) fp32.
"""

import os
import sys
import numpy as np
from math import factorial

for _p in ("/opt/trn_rl_repo",):
    if _p not in sys.path and os.path.isdir(_p):
        sys.path.insert(0, _p)

N_NODES, N_EDGES, FEAT = 512, 2048, 64
NCORES = 8
EC = N_EDGES // NCORES          # edges per core (256)
ECH = EC // 128                 # e-chunks of 128 per core (2)

F16 = True
DEBUG = False
NO_CC = False                   # replace collectives with local copies
STAGE = 6                       # build pipeline up to stage N (bisect helper)

# ---------------------------------------------------------------------------
# host-side math: real Wigner-3j tables (same construction as the model)
# ---------------------------------------------------------------------------

def _w3j_c(l1, l2, l3, m1, m2, m3):
    if m1 + m2 + m3 != 0:
        return 0.0
    f = factorial
    pref = ((-1.0) ** (l1 - l2 - m3)) * np.sqrt(
        f(l1 + l2 - l3) * f(l1 - l2 + l3) * f(-l1 + l2 + l3) / f(l1 + l2 + l3 + 1)
        * f(l1 + m1) * f(l1 - m1) * f(l2 + m2) * f(l2 - m2) * f(l3 + m3) * f(l3 - m3))
    s = 0.0
    for t in range(0, l1 + l2 - l3 + 1):
        ds = [t, l3 - l2 + t + m1, l3 - l1 + t - m2, l1 + l2 - l3 - t,
              l1 - t - m1, l2 - t + m2]
        if min(ds) < 0:
            continue
        den = 1
        for d in ds:
            den *= f(d)
        s += ((-1.0) ** t) / den
    return pref * s


def _u_real(l):
    U = np.zeros((2 * l + 1, 2 * l + 1), dtype=np.complex128)
    U[l, l] = 1.0
    for m in range(1, l + 1):
        U[l + m, l + m] = ((-1) ** m) / np.sqrt(2)
        U[l + m, l - m] = 1.0 / np.sqrt(2)
        U[l - m, l - m] = 1j / np.sqrt(2)
        U[l - m, l + m] = -1j * ((-1) ** m) / np.sqrt(2)
    return U


def _real_w3j(l1, l2, l3):
    W = np.zeros((2 * l1 + 1, 2 * l2 + 1, 2 * l3 + 1), dtype=np.complex128)
    for a, m1 in enumerate(range(-l1, l1 + 1)):
        for b, m2 in enumerate(range(-l2, l2 + 1)):
            for c, m3 in enumerate(range(-l3, l3 + 1)):
                W[a, b, c] = _w3j_c(l1, l2, l3, m1, m2, m3)
    C = np.einsum('am,bn,co,mno->abc', _u_real(l1), _u_real(l2), _u_real(l3), W)
    C = C.real + C.imag
    n = np.linalg.norm(C)
    if n > 0:
        C = C / n
    return C


W3J = {(a, b, c): _real_w3j(a, b, c)
       for a in range(3) for b in range(3) for c in range(3)
       if abs(a - b) <= c <= a + b}

SH_OFF = [0, 1, 4]
RELU_GAIN = float(np.sqrt(2.0))


def l3k_idx(l3, k):
    return l3 * l3 + k


def tp_instructions(in_ls):
    ins = []
    for i1, l1 in enumerate(in_ls):
        for l2 in range(3):
            for l3 in range(3):
                if abs(l1 - l2) <= l3 <= l1 + l2 and \
                        ((-1) ** (l1 + l2)) == (-1) ** l3:
                    ins.append((i1, l1, l2, l3))
    return ins


class ConvMeta:
    """Compile-time layout metadata for one equivariant conv layer."""

    def __init__(self, name, in_ls, mul, C, pair_t, expand=False):
        self.name, self.in_ls, self.mul, self.C, self.pair_t = \
            name, in_ls, mul, C, pair_t
        self.expand = expand
        self.ins = tp_instructions(in_ls)
        fan = {0: 0, 1: 0, 2: 0}
        for (_, l1, l2, l3) in self.ins:
            fan[l3] += mul
        self.fan = fan
        self.l1_groups = []
        for l1v in sorted(set(l1 for (_, l1, _, _) in self.ins)):
            idxs = [n for n, (_, l1x, _, _) in enumerate(self.ins) if l1x == l1v]
            self.l1_groups.append((l1v, idxs))
        # s-terms: (gi, gii, i, k, l3, jlist, clist); one Cmat column each
        self.sterms = []
        for gi, (l1v, idxs) in enumerate(self.l1_groups):
            for gii, n in enumerate(idxs):
                (_, l1x, l2x, l3x) = self.ins[n]
                Cw = W3J[(l1x, l2x, l3x)]
                alpha = np.sqrt(2 * l3x + 1) / np.sqrt(fan[l3x])
                for i in range(2 * l1x + 1):
                    for k in range(2 * l3x + 1):
                        jl, cl = [], []
                        for j in range(2 * l2x + 1):
                            c = Cw[i, j, k] * alpha
                            if abs(c) > 1e-12:
                                jl.append(SH_OFF[l2x] + j)
                                cl.append(float(c))
                        if jl:
                            self.sterms.append((gi, gii, i, k, l3x, jl, cl))
        self.blocks = [(l3, k) for l3 in range(3) for k in range(2 * l3 + 1)]
        self.Dout = len(self.blocks) * C
        self.nt = 6 if pair_t else 12
        # scatter stacking: how many (l3,k) blocks fit 128 output partitions
        self.sstack = max(1, 128 // C)
        # s-application plan: per (gi, i, gii) either one broadcast scale of
        # the z block over its nk terms (stride-0 read of PSUM) into a zs
        # staging tile + run-contiguous adds into msgall, or per-term fused
        # scalar ops -- whichever needs fewer instructions.
        # splan[(gi,i)] = (zs_width, ops); ops entries:
        #   ('scale', gii, nk, zs_pos, ncol0)
        #   ('add', zs_pos, bi0, nrun)
        #   ('term', gii, ncol, bi)
        # ncol* are indices into this conv's ordered scol list (see CMAT).
        self.splan = {}
        self.ordered_scols = []   # (jl, cl) in device order
        for gi, (l1v, idxs) in enumerate(self.l1_groups):
            for i in range(2 * l1v + 1):
                ops = []
                zsw = 0
                for gii in range(len(idxs)):
                    terms = [st for st in self.sterms
                             if st[0] == gi and st[2] == i and st[1] == gii]
                    terms.sort(key=lambda st: l3k_idx(st[4], st[3]))
                    if not terms:
                        continue
                    runs, prev = [], None
                    for p, st in enumerate(terms):
                        bi = l3k_idx(st[4], st[3])
                        if prev is not None and bi == prev + 1:
                            runs[-1][2] += 1
                        else:
                            runs.append([p, bi, 1])
                        prev = bi
                    ncol0 = len(self.ordered_scols)
                    for st in terms:
                        self.ordered_scols.append((st[5], st[6]))
                    nk = len(terms)
                    if 1 + len(runs) < nk:
                        ops.append(('scale', gii, nk, zsw, ncol0))
                        for (p, bi0, nrun) in runs:
                            ops.append(('add', zsw + p, bi0, nrun))
                        zsw += nk
                    else:
                        for p, st in enumerate(terms):
                            ops.append(('term', gii, ncol0 + p,
                                        l3k_idx(st[4], st[3])))
                self.splan[(gi, i)] = (zsw, ops)

    def w2slabs(self, w2):
        """w2 (12, W) -> list over l1-groups of slabs (nt, 128, nI*C) with the
        1/sqrt(12) radial norm folded in.  pair_t stacks (t=2g | t=2g+1) along
        the partition rows (mul=64)."""
        mul, C = self.mul, self.C
        woffs, off = [], 0
        for _ in self.ins:
            woffs.append(off)
            off += mul * C
        assert off == w2.shape[1]
        out = []
        for (l1v, idxs) in self.l1_groups:
            nI = len(idxs)
            slab = np.zeros((12, mul, nI * C), np.float64)
            for gii, n in enumerate(idxs):
                wi = w2[:, woffs[n]:woffs[n] + mul * C].reshape(12, mul, C)
                slab[:, :, gii * C:(gii + 1) * C] = wi
            slab = slab / np.sqrt(12.0)
            if self.pair_t:
                assert mul == 64
                slab = slab.reshape(6, 2, mul, nI * C).reshape(6, 128, nI * C)
            out.append(slab.astype(np.float16 if F16 else np.float32))
        return out

    def dev_cols(self):
        """Per-group per-t device slab widths."""
        return [len(idxs) * self.C for (l1v, idxs) in self.l1_groups]


CONVS = [
    ConvMeta('c1', [0], 128, 128, False),
    ConvMeta('c2', [0, 1, 2], 128, 64, False),
    ConvMeta('c3', [0, 1, 2], 64, 32, True),
]

# Global Cmat: columns in each conv's device (gi, i, gii, block-sorted)
# order so the scale columns per (gi,i,gii) are contiguous.
_SCOLS = []
for _cv in CONVS:
    _cv.scol_base = len(_SCOLS)
    _SCOLS.extend(_cv.ordered_scols)
NSCOL = len(_SCOLS)
CMAT = np.zeros((9, NSCOL), np.float32)
for _ci, (_jl, _cl) in enumerate(_SCOLS):
    for _j, _c in zip(_jl, _cl):
        CMAT[_j, _ci] = _c

BIDX = {}
for _l3 in range(3):
    for _k in range(2 * _l3 + 1):
        BIDX[(_l3, _k)] = _l3 * _l3 + _k


def xcols(mul):
    offs, off = {}, 0
    for l in range(3):
        for i in range(2 * l + 1):
            offs[(l, i)] = off
            off += mul
    return offs, off


XC2_OFF, XC2_D = xcols(128)     # 1152
XC3_OFF, XC3_D = xcols(64)      # 576

# ---------------------------------------------------------------------------
# blob layout: grouped regions, one DMA per group
#   each group = (rows, [(name, cols), ...]); host packs horizontally
# ---------------------------------------------------------------------------

_SLABCOLS = {cv.name: cv.dev_cols() for cv in CONVS}

B16_GROUPS = [
    ('g128', 128, [('ident16', 128),
                   ('siw1_0', 128), ('siw1_1', 128), ('siw1_2', 128)]
     + [(f'{cv.name}s{gi}', cv.nt * w)
        for cv in CONVS for gi, w in enumerate(_SLABCOLS[cv.name])]),
    ('g64', 64, [('featTd', EC), ('si0', 128),
                 ('siw2_0', 64), ('siw2_1', 64), ('siw2_2', 64)]),
    ('g32', 32, [('fsi0n', 32), ('siw3_0', 32), ('siw3_1', 32), ('siw3_2', 32)]),
    ('g12', 12, [('sel12', 12 * 128), ('sel3', 6 * 128)]),
    ('g11', 11, [('c1w1', 12), ('c2w1', 12), ('c3w1', 12)]),
    ('g9', 9, [('cmat', NSCOL)]),
    ('g1', 1, [('dstr', EC)]),
]
# fp32 group: posx(12) | srcf(ECH) | nlbbn(9) | vbias col(1)
B32_GROUPS = [
    ('f128', 128, [('posx', 12), ('srcf', ECH), ('nlbbn', 9), ('vbias', 1)]),
]


def _group_layout(groups):
    meta, regions, off = {}, {}, 0
    for gname, rows, entries in groups:
        w = sum(c for _, c in entries)
        regions[gname] = (off, rows, w)
        co = 0
        for name, cols in entries:
            meta[name] = (gname, co, rows, cols)
            co += cols
        off += -(-rows * w // 64) * 64
    return meta, regions, off


B16_META, B16_REG, B16_N = _group_layout(B16_GROUPS)
B32_META, B32_REG, B32_N = _group_layout(B32_GROUPS)

# ---------------------------------------------------------------------------
# host-side input preparation (sharding + constant baking)
# ---------------------------------------------------------------------------

def _prep_inputs(inputs):
    f16 = np.float16 if F16 else np.float32
    pos = np.asarray(inputs['pos'], np.float32)
    feats = np.asarray(inputs['features'], np.float32)
    ei = np.asarray(inputs['edge_index'])
    src = ei[0].astype(np.int64)
    dst = ei[1].astype(np.int64)

    def w1fold(w):
        return (np.asarray(w, np.float64) * RELU_GAIN /
                (1.12 * np.sqrt(11.0))).astype(f16)

    vals16 = {
        'cmat': CMAT.astype(f16),
        'ident16': np.eye(128, dtype=f16),
        'c1w1': w1fold(inputs['c1_rw1']),
        'c2w1': w1fold(inputs['c2_rw1']),
        'c3w1': w1fold(inputs['c3_rw1']),
        'si0': (np.asarray(inputs['si0_w'], np.float64) / np.sqrt(64.0)).astype(f16),
        'fsi0n': (-np.asarray(inputs['fsi_w'], np.float64)[0] / np.sqrt(32.0)).astype(f16),
    }
    sel12 = np.zeros((12, 12 * 128), f16)
    for t in range(12):
        sel12[t, t * 128:(t + 1) * 128] = 1.0
    sel3 = np.zeros((12, 6 * 128), f16)
    for g in range(6):
        sel3[2 * g, g * 128:g * 128 + 64] = 1.0
        sel3[2 * g + 1, g * 128 + 64:(g + 1) * 128] = 1.0
    vals16['sel12'] = sel12
    vals16['sel3'] = sel3

    for cv, key in zip(CONVS, ['c1_rw2', 'c2_rw2', 'c3_rw2']):
        for gi, slab in enumerate(cv.w2slabs(np.asarray(inputs[key], np.float64))):
            # conv2/conv3 receive x00 NEGATED (-softplus form); their l1=0
            # group consumes only x00, so fold the sign into that slab
            if cv.name in ('c2', 'c3') and gi == 0:
                slab = -slab
            vals16[f'{cv.name}s{gi}'] = np.ascontiguousarray(
                slab.transpose(1, 0, 2).reshape(slab.shape[1], -1))

    for li, (key, mul) in enumerate([('si1_w', 128), ('si2_w', 64), ('si3_w', 32)]):
        w = np.asarray(inputs[key], np.float64) / np.sqrt(mul)
        for l in range(3):
            vals16[f'siw{li + 1}_{l}'] = w[l].astype(f16)

    nlb = np.concatenate([np.asarray(inputs['nl1_b'], np.float32),
                          np.asarray(inputs['nl2_b'], np.float32),
                          np.asarray(inputs['nl3_b'], np.float32)])
    vb = np.zeros((128, 1), np.float32)
    vb[0:11, 0] = (-np.linspace(0.0, 8.0, 11) / 0.8).astype(np.float32)

    def pack(meta, regions, total, vals, dtype):
        blob = np.zeros(total, dtype)
        for name, (gname, co, rows, cols) in meta.items():
            goff, rows_, w = regions[gname]
            a = np.ascontiguousarray(vals[name], dtype)
            assert a.shape == (rows, cols), (name, a.shape, (rows, cols))
            region = blob[goff:goff + rows_ * w].reshape(rows_, w)
            region[:, co:co + cols] = a
        return blob.reshape(1, -1)

    in_maps = []
    for c in range(NCORES):
        sl = slice(c * EC, (c + 1) * EC)
        s_c, d_c = src[sl], dst[sl]
        v16 = dict(vals16)
        v16['featTd'] = np.ascontiguousarray(feats[d_c].T.astype(f16))
        v16['dstr'] = d_c.astype(f16).reshape(1, EC)
        posx = np.concatenate([pos[s_c].reshape(ECH, 128, 3).transpose(1, 0, 2)
                               .reshape(128, ECH * 3),
                               pos[d_c].reshape(ECH, 128, 3).transpose(1, 0, 2)
                               .reshape(128, ECH * 3)], axis=1)
        v32 = {
            'posx': posx.astype(np.float32),
            'srcf': np.ascontiguousarray(
                s_c.astype(np.float32).reshape(ECH, 128).T),
            'nlbbn': np.broadcast_to(-nlb.reshape(1, 9), (128, 9)).copy(),
            'vbias': vb,
        }
        in_maps.append({'blob16': pack(B16_META, B16_REG, B16_N, v16, np.float16),
                        'blob32': pack(B32_META, B32_REG, B32_N, v32, np.float32)})
    return in_maps


# ---------------------------------------------------------------------------
# device program
# ---------------------------------------------------------------------------

_CACHED = {}


def _build_program():
    import concourse.bass as bass
    import concourse.mybir as mybir
    from concourse import tile

    dt = mybir.dt
    AF = mybir.ActivationFunctionType
    ALU = mybir.AluOpType
    f16d = dt.float16 if F16 else dt.float32

    nc = bass.Bass("TRN2", target_bir_lowering=False, debug=False,
                   num_devices=1 if NO_CC else NCORES)

    B16 = nc.dram_tensor("blob16", [1, B16_N], f16d, kind="ExternalInput").ap()
    B32 = nc.dram_tensor("blob32", [1, B32_N], dt.float32,
                         kind="ExternalInput").ap()
    OUT = nc.dram_tensor("out", [N_NODES, 32], dt.float32,
                         kind="ExternalOutput").ap()

    with tile.TileContext(nc) as tc:
        with (
            tc.tile_pool(name="const", bufs=1) as cpool,
            tc.tile_pool(name="work", bufs=2) as wpool,
            tc.tile_pool(name="big", bufs=1) as bpool,
            tc.tile_pool(name="persist", bufs=1) as ppool,
            tc.tile_pool(name="psum", bufs=3, space="PSUM") as pmm,
            tc.tile_pool(name="psumtp", bufs=2, space="PSUM") as ptp,
            tc.tile_pool(name="dram", bufs=1, space="DRAM") as dpool,
        ):
            # ---- grouped const loads: one DMA per region ----
            gtiles = {}
            for gname, rows, entries in B16_GROUPS:
                off, rows_, w = B16_REG[gname]
                t = cpool.tile([rows, w], f16d, tag=gname)
                nc.sync.dma_start(out=t[:], in_=B16[0:1, off:off + rows * w]
                                  .rearrange("o (p c) -> (o p) c", p=rows))
                gtiles[gname] = t
            for gname, rows, entries in B32_GROUPS:
                off, rows_, w = B32_REG[gname]
                t = cpool.tile([rows, w], dt.float32, tag=gname)
                nc.sync.dma_start(out=t[:], in_=B32[0:1, off:off + rows * w]
                                  .rearrange("o (p c) -> (o p) c", p=rows))
                gtiles[gname] = t

            def cslice(name):
                gname, co, rows, cols = (B16_META[name] if name in B16_META
                                         else B32_META[name])
                return gtiles[gname][0:rows, co:co + cols]

            ident16 = cslice('ident16')
            cmat = cslice('cmat')
            vbias = cslice('vbias')[0:11, :]
            sel12 = cslice('sel12')
            sel3 = cslice('sel3')
            featTd = cslice('featTd')
            dstr = cslice('dstr')
            w1f = {1: cslice('c1w1'), 2: cslice('c2w1'), 3: cslice('c3w1')}
            si0 = cslice('si0')
            fsi0n = cslice('fsi0n')
            siw = {(li, l): cslice(f'siw{li}_{l}')
                   for li in (1, 2, 3) for l in range(3)}
            slabs = {}
            for cv in CONVS:
                for gi, cols in enumerate(_SLABCOLS[cv.name]):
                    slabs[(cv.name, gi)] = (cslice(f'{cv.name}s{gi}'),
                                            cv.nt, cols)
            posx = cslice('posx')
            srcf = cslice('srcf')
            nlbbn = cslice('nlbbn')

            ones16 = cpool.tile([1, 128], f16d, tag="ones16")
            nc.vector.memset(ones16[:], 1.0)
            onescol16 = cpool.tile([128, 1], f16d, tag="onescol16")
            nc.vector.memset(onescol16[:], 1.0)
            negone = cpool.tile([128, 1], f16d, tag="negone")
            nc.vector.memset(negone[:], -1.0)
            eps24 = cpool.tile([128, 1], dt.float32, tag="eps24")
            nc.vector.memset(eps24[:], 1e-24)

            # ---------------- S incidence ----------------
            iota = ppool.tile([128, N_NODES], dt.float32, tag="iota")
            nc.gpsimd.iota(iota[:], pattern=[[1, N_NODES]], base=0,
                           channel_multiplier=0,
                           allow_small_or_imprecise_dtypes=True)
            S = []
            for ec in range(ECH):
                st = ppool.tile([128, N_NODES], f16d, tag=f"S{ec}")
                nc.vector.tensor_scalar(st[:], iota[:], srcf[:, ec:ec + 1], None,
                                        ALU.is_equal)
                S.append(st)

            # Sdst[nch]: (128 nodes, EC) one-hot of dst for the gather matmul
            dstb_ps = pmm.tile([128, EC], dt.float32, tag="mm")
            nc.tensor.matmul(dstb_ps[:], ones16[:], dstr[:], start=True, stop=True)
            dstb = ppool.tile([128, EC], f16d, tag="dstb")
            nc.scalar.copy(out=dstb[:], in_=dstb_ps[:])
            Sdst = []
            for nch in range(4):
                nio = ppool.tile([128, 1], dt.float32, tag=f"nio{nch}")
                nc.gpsimd.iota(nio[:], pattern=[[1, 1]], base=nch * 128,
                               channel_multiplier=1,
                               allow_small_or_imprecise_dtypes=True)
                sd = ppool.tile([128, EC], f16d, tag=f"Sdst{nch}")
                nc.vector.tensor_scalar(sd[:], dstb[:], nio[:], None,
                                        ALU.is_equal)
                Sdst.append(sd)

            # ---------------- edge scalars (both chunks paired) ----------------
            s3c, s15c, s5c = float(np.sqrt(3.0)), float(np.sqrt(15.0)), float(np.sqrt(5.0))
            vec = wpool.tile([128, 2 * 3], dt.float32, tag="vec")
            nc.vector.tensor_sub(vec[:], posx[:, 0:6], posx[:, 6:12])
            vsq = wpool.tile([128, 6], dt.float32, tag="vsq")
            nc.vector.tensor_mul(vsq[:], vec[:], vec[:])
            d2 = wpool.tile([128, 2], dt.float32, tag="d2")
            nc.vector.tensor_reduce(d2[:].rearrange("p (a o) -> p a o", o=1),
                                    vsq[:].rearrange("p (a c) -> p a c", c=3),
                                    mybir.AxisListType.X, ALU.add)
            dist = ppool.tile([128, 2], dt.float32, tag="dist")
            nc.scalar.sqrt(dist[:], d2[:])
            dmax = wpool.tile([128, 2], dt.float32, tag="dmax")
            nc.vector.tensor_scalar_max(dmax[:], dist[:], 1e-12)
            dinv = wpool.tile([128, 2], dt.float32, tag="dinv")
            nc.vector.reciprocal(dinv[:], dmax[:])
            dirs = wpool.tile([128, 6], dt.float32, tag="dirs")
            nc.vector.tensor_mul(
                dirs[:].rearrange("p (a c) -> p a c", c=3),
                vec[:].rearrange("p (a c) -> p a c", c=3),
                dinv[:].rearrange("p (a c) -> p a c", c=1).to_broadcast([128, 2, 3]))

            def dpair(axis):  # [128, 2] view of (dx|dy|dz) for both chunks
                return dirs[:].rearrange("p (a c) -> p a c", c=3)[:, :, axis:axis + 1] \
                    .rearrange("p a c -> p (a c)")

            dx, dy, dz = dpair(0), dpair(1), dpair(2)
            # sh18: j-major pairs [128, (j, ec)]
            sh18 = ppool.tile([128, 18], f16d, tag="sh18")
            nc.vector.memset(sh18[:, 0:2], 1.0)
            nc.vector.tensor_scalar_mul(sh18[:, 2:4], dy, s3c)
            nc.vector.tensor_scalar_mul(sh18[:, 4:6], dz, s3c)
            nc.vector.tensor_scalar_mul(sh18[:, 6:8], dx, s3c)
            tmp = wpool.tile([128, 2], dt.float32, tag="shtmp")
            tmp2 = wpool.tile([128, 2], dt.float32, tag="shtmp2")
            nc.vector.tensor_mul(tmp[:], dx, dy)
            nc.vector.tensor_scalar_mul(sh18[:, 8:10], tmp[:], s15c)
            nc.vector.tensor_mul(tmp[:], dy, dz)
            nc.vector.tensor_scalar_mul(sh18[:, 10:12], tmp[:], s15c)
            nc.vector.tensor_mul(tmp[:], dz, dz)
            nc.vector.tensor_scalar(sh18[:, 12:14], tmp[:], 3.0 * s5c / 2.0,
                                    -s5c / 2.0, ALU.mult, ALU.add)
            nc.vector.tensor_mul(tmp[:], dx, dz)
            nc.vector.tensor_scalar_mul(sh18[:, 14:16], tmp[:], s15c)
            nc.vector.tensor_mul(tmp[:], dx, dx)
            nc.vector.tensor_mul(tmp2[:], dy, dy)
            nc.vector.tensor_sub(tmp[:], tmp[:], tmp2[:])
            nc.vector.tensor_scalar_mul(sh18[:, 16:18], tmp[:], s15c / 2.0)

            # smat = sh @ CMAT per e-chunk (e on partitions)
            smat = []
            for ec in range(ECH):
                shT_ps = ptp.tile([9, 128], f16d, tag="tp16")
                shv = sh18[:].rearrange("p (j e) -> p j e", e=2)[:, :, ec:ec + 1] \
                    .rearrange("p j e -> p (j e)")
                nc.tensor.transpose(shT_ps[:], shv, ident16[:])
                shT = wpool.tile([9, 128], f16d, tag="shT")
                nc.scalar.copy(out=shT[:], in_=shT_ps[:])
                sm_ps = pmm.tile([128, NSCOL], dt.float32, tag="mm")
                nc.tensor.matmul(sm_ps[:], shT[:], cmat[:], start=True, stop=True)
                sm = ppool.tile([128, NSCOL], dt.float32, tag=f"smat{ec}")
                nc.vector.tensor_copy(sm[:], sm_ps[:])
                smat.append(sm)

            # radial basis row + per-conv hT
            d16 = wpool.tile([128, 2], f16d, tag="d16")
            nc.vector.tensor_copy(d16[:], dist[:])
            distr = ppool.tile([1, EC], f16d, tag="distr")
            for ec in range(ECH):
                dr_ps = ptp.tile([1, 128], f16d, tag="tp16")
                nc.tensor.transpose(dr_ps[:], d16[:, ec:ec + 1], ident16[:])
                nc.scalar.copy(out=distr[:, ec * 128:(ec + 1) * 128],
                               in_=dr_ps[:])
            db_ps = pmm.tile([11, EC], dt.float32, tag="mm")
            nc.tensor.matmul(db_ps[:], ones16[:, 0:11], distr[:],
                             start=True, stop=True)
            step = 0.8
            sqt = wpool.tile([11, EC], dt.float32, tag="sqt")
            nc.scalar.activation(sqt[:], db_ps[:], AF.Square,
                                 bias=vbias[:], scale=1.0 / step)
            rb = ppool.tile([11, EC], f16d, tag="rb")
            nc.scalar.activation(rb[:], sqt[:], AF.Exp, scale=-1.0)
            hT = {}
            for cvi, cv in enumerate(CONVS):
                h_ps = pmm.tile([12, EC], dt.float32, tag="mm")
                nc.tensor.matmul(h_ps[:], w1f[cvi + 1][:], rb[:],
                                 start=True, stop=True)
                ht = ppool.tile([12, EC], f16d, tag=f"hT{cv.name}")
                nc.scalar.activation(ht[:], h_ps[:], AF.Relu)
                hT[cv.name] = ht

            # per-conv radial rows broadcast to all 128 partitions, all t
            # side by side: hbcat[p, (t, e)] = h_t[e]
            hbcats = {}
            for cv in CONVS:
                sel = sel3 if cv.pair_t else sel12
                hc = bpool.tile([128, cv.nt * EC], f16d, tag=f"hbc{cv.name}")
                for t in range(cv.nt):
                    hb_ps = pmm.tile([128, EC], dt.float32, tag="mm")
                    nc.tensor.matmul(hb_ps[:], sel[:, t * 128:(t + 1) * 128],
                                     hT[cv.name][:], start=True, stop=True)
                    nc.scalar.copy(out=hc[:, t * EC:(t + 1) * EC], in_=hb_ps[:])
                hbcats[cv.name] = hc

            # conv1 input block: x1T = si0.T @ features[dst].T
            x1_ps = pmm.tile([128, EC], dt.float32, tag="mm")
            nc.tensor.matmul(x1_ps[:], si0[:], featTd[:], start=True, stop=True)
            x1T_c1 = ppool.tile([128, EC], f16d, tag="x1Tc1")
            nc.scalar.copy(out=x1T_c1[:], in_=x1_ps[:])

            # ---------------- conv driver ----------------
            def run_conv(cv, x1T_groups, arin, arout,
                         stop_before_scatter=False):
                name, C = cv.name, cv.C
                nt = cv.nt
                hbcat = hbcats[name]
                # msgall[ec]: all (l3,k) message blocks as column slices.
                # The two e-chunks' chains are independent; ec=0 runs on the
                # vector engine, ec=1 on gpsimd (Pool) for queue parallelism.
                eng = [nc.vector, nc.gpsimd]
                msgall = []
                for ec in range(ECH):
                    ma = ppool.tile([128, 9 * C], f16d, tag=f"msga{ec}",
                                    name=f"msga_{name}_{ec}")
                    eng[ec].memset(ma[:], 0.0)
                    msgall.append(ma)
                for gi, (l1v, idxs) in enumerate(cv.l1_groups):
                    ni = 2 * l1v + 1
                    nI = len(idxs)
                    x1g = x1T_groups[l1v]
                    slab_t, s_nt, s_cols = slabs[(name, gi)]
                    assert s_nt == nt and s_cols == nI * C
                    for i in range(ni):
                        zsw, ops = cv.splan[(gi, i)]
                        for ec in range(ECH):
                            ve = eng[ec]
                            # G for this (i, ec): all t-blocks in one mul
                            gie = wpool.tile([128, nt * 128], f16d,
                                             tag=f"Gie{ec}")
                            ve.tensor_mul(
                                gie[:].rearrange("p (t e) -> p t e", e=128),
                                x1g[:, i * EC + ec * 128:i * EC + (ec + 1) * 128]
                                .rearrange("p (t e) -> p t e", t=1)
                                .to_broadcast([128, nt, 128]),
                                hbcat[:].rearrange("p (t e) -> p t e", e=EC)
                                [:, :, ec * 128:(ec + 1) * 128])
                            z_ps = pmm.tile([128, nI * C], dt.float32, tag="mm")
                            for t in range(nt):
                                nc.tensor.matmul(
                                    z_ps[:],
                                    gie[:, t * 128:(t + 1) * 128],
                                    slab_t[:, t * s_cols:(t + 1) * s_cols],
                                    start=(t == 0), stop=(t == nt - 1))
                            if ec == 1:
                                # gpsimd cannot read PSUM: bounce z to SBUF
                                z_sb = wpool.tile([128, nI * C], f16d,
                                                  tag="zsb")
                                nc.scalar.copy(out=z_sb[:], in_=z_ps[:])
                                z_rd = z_sb
                            else:
                                z_rd = z_ps
                            zs = None
                            if zsw:
                                zs = wpool.tile([128, zsw * C], f16d,
                                                tag=f"zs{ec}")
                            for op in ops:
                                if op[0] == 'scale':
                                    _, gii, nk, zp, nc0 = op
                                    sc0 = cv.scol_base + nc0
                                    ve.tensor_mul(
                                        zs[:, zp * C:(zp + nk) * C]
                                        .rearrange("p (a c) -> p a c", c=C),
                                        z_rd[:, gii * C:(gii + 1) * C]
                                        .rearrange("p (a c) -> p a c", a=1)
                                        .to_broadcast([128, nk, C]),
                                        smat[ec][:, sc0:sc0 + nk]
                                        .rearrange("p (a c) -> p a c", c=1)
                                        .to_broadcast([128, nk, C]))
                                elif op[0] == 'add':
                                    _, zp, bi0, nrun = op
                                    dst_sl = msgall[ec][:, bi0 * C:
                                                        (bi0 + nrun) * C]
                                    ve.tensor_add(
                                        dst_sl, dst_sl,
                                        zs[:, zp * C:(zp + nrun) * C])
                                else:
                                    _, gii, ncol, bi = op
                                    sci = cv.scol_base + ncol
                                    sc = smat[ec][:, sci:sci + 1]
                                    dst_sl = msgall[ec][:, bi * C:(bi + 1) * C]
                                    zsl = z_rd[:, gii * C:(gii + 1) * C]
                                    nc.vector.scalar_tensor_tensor(
                                        dst_sl, zsl, sc, dst_sl,
                                        ALU.mult, ALU.add)
                if stop_before_scatter:
                    return {}
                # scatter: stack sstack blocks along output partitions per
                # matmul; staged in stacked [(q,C), (g,n)] form (PSUM reads
                # must start at partition 0), de-stacked by the DMA pattern
                ss = cv.sstack
                ngr = -(-9 // ss)
                stage = bpool.tile([ss * C, ngr * N_NODES], f16d,
                                   tag="aggstage")
                if ngr * ss > 9:
                    # zero the whole stage so the padded tail rows of the
                    # last group are defined (partition-offset writes are
                    # restricted; a full memset is one op)
                    nc.vector.memset(stage[:], 0.0)
                for g in range(ngr):
                    b0 = g * ss
                    nb_ = min(ss, 9 - b0)
                    agg_ps = pmm.tile([nb_ * C, N_NODES], dt.float32, tag="mm")
                    for ec in range(ECH):
                        nc.tensor.matmul(agg_ps[:],
                                         msgall[ec][:, b0 * C:(b0 + nb_) * C],
                                         S[ec][:],
                                         start=(ec == 0), stop=(ec == ECH - 1))
                    nc.scalar.copy(
                        out=stage[0:nb_ * C, g * N_NODES:(g + 1) * N_NODES],
                        in_=agg_ps[:])
                assert ss * C == 128
                nc.sync.dma_start(
                    out=arin.rearrange("(g p) n -> p g n", p=128),
                    in_=stage[:].rearrange("p (g n) -> p g n", n=N_NODES))
                if NO_CC:
                    nc.sync.dma_start(out=arout[:, :], in_=arin[:, :])
                else:
                    nc.gpsimd.collective_compute(
                        "AllReduce", ALU.add,
                        replica_groups=[list(range(NCORES))],
                        ins=[arin.opt()], outs=[arout.opt()])
                agg_all = ppool.tile([C, 9 * N_NODES], f16d, tag="aggall")
                nc.sync.dma_start(
                    out=agg_all[:].rearrange("c (b n) -> c b n", b=9),
                    in_=arout[0:9 * C, :].rearrange("(b c) n -> c b n", b=9))
                return agg_all

            def node_phase(cv_idx, agg_all, Cblk, mul_out, last=False):
                blocks = [(l, k) for l in range(3) for k in range(2 * l + 1)]

                def ablk(l, k):
                    bi = BIDX[(l, k)]
                    return agg_all[:, bi * N_NODES:(bi + 1) * N_NODES]

                sq_all = wpool.tile([Cblk, 9 * N_NODES], f16d, tag="sqall")
                nc.vector.tensor_mul(sq_all[:], agg_all[:], agg_all[:])
                ss_ps = pmm.tile([1, N_NODES], dt.float32, tag="mm")
                for bi in range(9):
                    nc.tensor.matmul(ss_ps[:], onescol16[0:Cblk, :],
                                     sq_all[:, bi * N_NODES:(bi + 1) * N_NODES],
                                     start=(bi == 0), stop=(bi == 8))
                sroot = wpool.tile([1, N_NODES], dt.float32, tag="sroot")
                nc.scalar.sqrt(sroot[:], ss_ps[:])
                nc.vector.tensor_scalar_add(sroot[:], sroot[:], 1e-6)
                nfi = wpool.tile([1, N_NODES], dt.float32, tag="nfi")
                nc.vector.reciprocal(nfi[:], sroot[:])
                # clamp so empty-aggregate nodes (1/1e-6) stay fp16-finite
                nfi16 = wpool.tile([1, N_NODES], f16d, tag="nfi16")
                nc.vector.tensor_scalar_min(nfi16[:], nfi[:], 60000.0)
                nb_ps = pmm.tile([128, N_NODES], dt.float32, tag="mm")
                nc.tensor.matmul(nb_ps[:], ones16[:], nfi16[:],
                                 start=True, stop=True)
                nb = bpool.tile([128, N_NODES], f16d, tag="nb")
                nc.scalar.copy(out=nb[:], in_=nb_ps[:])
                rhsn_all = bpool.tile([Cblk, 9 * N_NODES], f16d, tag="rhsnall")
                nc.vector.tensor_mul(
                    rhsn_all[:].rearrange("c (b n) -> c b n", n=N_NODES),
                    agg_all[:].rearrange("c (b n) -> c b n", n=N_NODES),
                    nb[0:Cblk, :].rearrange("c (b n) -> c b n", b=1)
                    .to_broadcast([Cblk, 9, N_NODES]))
                v = {}
                use_blocks = [(0, 0)] if last else blocks
                for (l, k) in use_blocks:
                    bi = BIDX[(l, k)]
                    si_ps = pmm.tile([mul_out, N_NODES], dt.float32, tag="mm")
                    nc.tensor.matmul(si_ps[:], siw[(cv_idx, l)][:],
                                     rhsn_all[:, bi * N_NODES:(bi + 1) * N_NODES],
                                     start=True, stop=True)
                    vt = ppool.tile([mul_out, N_NODES], f16d, tag=f"v_{l}_{k}")
                    nc.scalar.copy(out=vt[:], in_=si_ps[:])
                    v[(l, k)] = vt

                def nsoftplus(out_ap, in_ap, bias_ap, P):
                    # out = ln(sigmoid(-(x+b))) = -softplus(x+b)
                    s = wpool.tile([P, N_NODES], dt.float32, tag="sps")
                    nc.scalar.activation(s[:], in_ap, AF.Sigmoid,
                                         bias=bias_ap, scale=-1.0)
                    nc.scalar.activation(out_ap, s[:], AF.Ln)

                x = {}
                bcol = 3 * (cv_idx - 1)
                # x0n = -softplus(v00 + b); sign folded into consumers
                x0n = ppool.tile([mul_out, N_NODES], f16d, tag="x_0_0")
                nsoftplus(x0n[:], v[(0, 0)][:],
                          nlbbn[0:mul_out, bcol:bcol + 1], mul_out)
                x[(0, 0)] = x0n
                if last:
                    return x
                for l in (1, 2):
                    ssq = wpool.tile([mul_out, N_NODES], f16d, tag="nlssq")
                    nc.vector.tensor_mul(ssq[:], v[(l, 0)][:], v[(l, 0)][:])
                    for k in range(1, 2 * l + 1):
                        sq2 = wpool.tile([mul_out, N_NODES], f16d, tag="nlsq2")
                        nc.vector.tensor_mul(sq2[:], v[(l, k)][:], v[(l, k)][:])
                        nc.vector.tensor_add(ssq[:], ssq[:], sq2[:])
                    groot = wpool.tile([mul_out, N_NODES], f16d, tag="groot")
                    nc.scalar.activation(groot[:], ssq[:], AF.Sqrt,
                                         bias=eps24[0:mul_out, :])
                    gaten = wpool.tile([mul_out, N_NODES], f16d, tag="gate")
                    nsoftplus(gaten[:], groot[:],
                              nlbbn[0:mul_out, bcol + l:bcol + l + 1], mul_out)
                    for k in range(2 * l + 1):
                        xt = ppool.tile([mul_out, N_NODES], f16d,
                                        tag=f"x_{l}_{k}")
                        # x = v * softplus = (gaten * -1) * v
                        nc.vector.scalar_tensor_tensor(
                            xt[:], gaten[:], negone[0:mul_out, :], v[(l, k)][:],
                            ALU.mult, ALU.mult)
                        x[(l, k)] = xt
                return x

            def assemble_and_gather(x, mul, xoff, double_rows):
                # xrow[nch]: [128 nodes, D] node-major; x00 negated via identn
                D = 9 * mul
                xrow = []
                for nch in range(4):
                    xr = bpool.tile([128, D], f16d, tag=f"xrow{nch}")
                    xrow.append(xr)
                for (l, k), blk in x.items():
                    co = xoff[(l, k)]
                    for nch in range(4):
                        # XBAR DMA transpose: [mul, 128] -> [128, mul]
                        issuer = nc.sync if nch % 2 == 0 else nc.scalar
                        issuer.dma_start(
                            out=xrow[nch][:, co:co + mul],
                            in_=blk[:, nch * 128:(nch + 1) * 128],
                            transpose=True)
                # x1g[l][u, i*EC+e] = x[dst_e][(l,i) block, u] via incidence MMs
                x1g = {}
                for l in range(3):
                    ni = 2 * l + 1
                    xt = ppool.tile([128, ni * EC], f16d, tag=f"x1g{l}")
                    for i in range(ni):
                        co = xoff[(l, i)]
                        g_ps = pmm.tile([mul, EC], dt.float32, tag="mm")
                        for nch in range(4):
                            nc.tensor.matmul(g_ps[:],
                                             xrow[nch][:, co:co + mul],
                                             Sdst[nch][:],
                                             start=(nch == 0), stop=(nch == 3))
                        nc.scalar.copy(out=xt[0:mul, i * EC:(i + 1) * EC],
                                       in_=g_ps[:])
                        if double_rows:
                            nc.scalar.copy(out=xt[64:128, i * EC:(i + 1) * EC],
                                           in_=g_ps[:])
                    x1g[l] = xt
                return x1g

            def ar_rows(cv):
                ss = cv.sstack
                return -(-9 // ss) * ss * cv.C

            ar1_in = dpool.tile([ar_rows(CONVS[0]), N_NODES], f16d, tag="ar1in")
            ar1_out = dpool.tile([ar_rows(CONVS[0]), N_NODES], f16d, tag="ar1out", addr_space="Shared")
            ar2_in = dpool.tile([ar_rows(CONVS[1]), N_NODES], f16d, tag="ar2in")
            ar2_out = dpool.tile([ar_rows(CONVS[1]), N_NODES], f16d, tag="ar2out", addr_space="Shared")
            ar3_in = dpool.tile([ar_rows(CONVS[2]), N_NODES], f16d, tag="ar3in")
            ar3_out = dpool.tile([ar_rows(CONVS[2]), N_NODES], f16d, tag="ar3out", addr_space="Shared")

            done = False
            if STAGE >= 2:
                agg1 = run_conv(CONVS[0], {0: x1T_c1}, ar1_in[:], ar1_out[:],
                                stop_before_scatter=(STAGE == 2))
            if STAGE >= 3:
                x2 = node_phase(1, agg1, CONVS[0].C, 128)
            if STAGE >= 4:
                x1g2 = assemble_and_gather(x2, 128, XC2_OFF, False)
            if STAGE >= 5:
                agg2 = run_conv(CONVS[1], x1g2, ar2_in[:], ar2_out[:])
                x3 = node_phase(2, agg2, CONVS[1].C, 64)
                x1g3 = assemble_and_gather(x3, 64, XC3_OFF, True)
            if STAGE >= 6:
                agg3 = run_conv(CONVS[2], x1g3, ar3_in[:], ar3_out[:])
                x4 = node_phase(3, agg3, CONVS[2].C, 32, last=True)

                # out = fsi0 @ softplus = (-fsi0) @ x0n  (sign pre-folded)
                fp_ps = pmm.tile([32, N_NODES], dt.float32, tag="mm")
                nc.tensor.matmul(fp_ps[:], fsi0n[:], x4[(0, 0)][:],
                                 start=True, stop=True)
                fs = wpool.tile([32, N_NODES], f16d, tag="fs")
                nc.scalar.copy(out=fs[:], in_=fp_ps[:])
                otall = wpool.tile([128, 4 * 32], dt.float32, tag="otall")
                for nch in range(4):
                    ot_ps = ptp.tile([128, 128], f16d, tag="tp16")
                    nc.tensor.transpose(ot_ps[0:128, 0:32],
                                        fs[:, nch * 128:(nch + 1) * 128],
                                        ident16[0:32, 0:32])
                    nc.vector.tensor_copy(otall[:, nch * 32:(nch + 1) * 32],
                                          ot_ps[0:128, 0:32])
                nc.sync.dma_start(
                    out=OUT[:].rearrange("(n p) c -> p n c", p=128),
                    in_=otall[:].rearrange("p (n c) -> p n c", n=4))
                done = True
            if not done:
                otd = wpool.tile([128, 4 * 32], dt.float32, tag="otdummy")
                nc.vector.memset(otd[:], 0.0)
                nc.sync.dma_start(
                    out=OUT[:].rearrange("(n p) c -> p n c", p=128),
                    in_=otd[:].rearrange("p (n c) -> p n c", n=4))

    return nc


_NOSPLIT_TYPES = {
    'InstNoOp', 'InstEventSemaphore',
    'InstUnconditionalBranch', 'InstConditionalBranch', 'InstHalt',
    'InstRegisterMove', 'InstPseudoReloadLibraryIndex',
}


def _split_waits(nc):
    """Walrus in this toolchain allows only one sync-wait slot on compute
    ISA instructions; hoist extra waits onto a same-engine NoOp placed
    immediately before."""
    import concourse.mybir as mybir
    nsplit = 0
    for bb in nc.main_func.blocks:
        out = []
        for ins in bb.instructions:
            si = ins.sync_info
            if (si is not None and si.on_wait and len(si.on_wait) > 1
                    and type(ins).__name__ not in _NOSPLIT_TYPES):
                for wi, w in enumerate(si.on_wait[:-1]):
                    nop = mybir.InstNoOp(name=f"{ins.name}-ws{wi}",
                                         ins=[], outs=[])
                    nop.engine = ins.engine
                    nop.sync_info = mybir.SyncInfo(on_wait=[w], on_update=[])
                    out.append(nop)
                ins.sync_info = mybir.SyncInfo(on_wait=list(si.on_wait[-1:]),
                                               on_update=si.on_update)
                nsplit += 1
            out.append(ins)
        bb.instructions[:] = out
    return nsplit


def get_program(split=True):
    key = ('nc', split)
    if key not in _CACHED:
        nc = _build_program()
        if split:
            _split_waits(nc)
        _CACHED[key] = nc
    return _CACHED[key]


def kernel(**inputs):
    in_maps = _prep_inputs(inputs)
    nc = get_program()
    from concourse import bass_utils
    os.environ['BASS_NEVER_TRACE'] = '1'
    res = bass_utils.run_bass_kernel_spmd(nc, in_maps,
                                          core_ids=list(range(NCORES)))
    return np.asarray(res.results[0]['out'], np.float32)
